# revision 18
# baseline (speedup 1.0000x reference)
"""MoE dispatcher kernel for Trainium2 (8 NeuronCores, expert-parallel).

Contract: kernel(**inputs) takes FULL inputs and returns the FULL output.

Strategy (expert-parallel, matches the sharding hint):
  - host: softmax(gate_logits) -> top-2 -> combine weights per (token, expert)
  - host "all-to-all dispatch": for expert e, gather its routed tokens,
    pre-scale rows by the combine weight (w * (x @ W) == (w*x) @ W), pad to a
    common capacity C, transpose to [D, C] so the device streams tokens along
    the free dim.  One expert per core.
  - device (per core): Y^T[D,C] = W[e]^T @ X^T via PE array, tiled
    [128 x <=512] PSUM accumulation over K=D.
  - host "all-to-all combine": scatter-add each expert's Y rows back to the
    token axis (plain add; weights were folded into x).

Measured structure (ntff forensics, 49.0us baseline):
  - exec_time_ns = (last teardown instruction end) - (first framework MEMSET).
    The ~5.9us runtime preamble before the memset is NOT counted; the ~7.2us
    teardown (all 254 semaphores cleared one inst each, engine-partitioned,
    Tensor's 53-clear chain at 115ns each is critical) IS counted and is
    runtime-fixed.
  - The teardown chain rate is clock-gate independent (115ns/clear at both
    K=4/8 and 8/8), so tail dummy matmuls only push the end barrier out;
    TAIL_MM now defaults 0.
  - Head: engines exit the framework preamble ~7.0-7.2us.  DMA triggers take
    ~0.65us of descriptor-gen each and the ring adds ~1.3us latency, so the
    first input bytes land ~8.7us.  v1 serialized all W chunks on one queue
    and memset the warm tile first: first real MM at ~12.4us, HAM clock warm
    only at ~13.9us.
  - This version: warmup matmuls are 128-wide (107ns cold granularity), read
    uninitialized SBUF (dead PSUM; every real accumulation starts with
    start=True so leftover garbage never matters), and start at ~7.2us.  The
    critical first-pass data (W k=0 halves + x0 chunk 0) rides three engines'
    queues in parallel so descriptor generation overlaps; first real MM ~9.5us.

DRAM layouts are host-permuted so every DMA is fully contiguous per
partition:
  w   [P, KT*D]            w[p, k*D + d] = W[e][k*128 + p, d]
  xt  [NT, P, KT*NSPLIT]   xt[j, p, k*nsz_j + n] = X^T[k*128 + p, n0_j + n]
  yt  [MT, P, C]           yt[m, p, n]   = Y^T[m*128 + p, n]
"""

import os

import numpy as np

N_CORES = 8
P = 128
NSPLIT = 512  # max moving-operand / PSUM-bank free dim (fp32)
NMIN = 256  # keep moving tiles >=256 wide (per-MM floor ~107ns below that)
PASSES = 8  # k-dim chunks for W / first-n-tile pipelining

MM_DT = os.environ.get("BASS_MOE_DT", "bfloat16")
OUT_DT = os.environ.get("BASS_MOE_OUT_DT", "bfloat16")
# 128-wide warmup matmuls bridging preamble-end past the first-data arrival
# (W-c0 + x0-c0 = 512KB at full ring rate ~ memset0+4.6us).  Cores whose data
# lands early pay ~56ns per excess warmup; a core whose data is late but
# whose warmups ended early idles >3.4us and HAM re-throttles its whole
# stream (v4 core6: +1.7us).  The graded metric is the max core.
WARMUP_MM = int(os.environ.get("BASS_MOE_WARMUP", "34"))
# dummy matmuls after the last real one (teardown chain is clock-independent,
# so these only delay the end barrier; 0 unless measurement says otherwise)
TAIL_MM = int(os.environ.get("BASS_MOE_TAIL", "0"))
OUT_Q = os.environ.get("BASS_MOE_OUT_Q", "alt")
NT_MODE = os.environ.get("BASS_MOE_NT", "even")
C_MODE = os.environ.get("BASS_MOE_C", "exact")
# the warm tile is only [P, 128] now, so its memset is ~0.1us (the tile
# scheduler requires every read tile to have a writer, so 0 is not allowed)
WARM_INIT = int(os.environ.get("BASS_MOE_WARM_INIT", "1"))

_prog_cache: dict = {}


def _np_dt(name):
    if name == "bfloat16":
        import ml_dtypes

        return ml_dtypes.bfloat16
    return np.float32


def _n_tiles(C):
    """Split C into tiles of at most NSPLIT (remainder split evenly)."""
    out = []
    rem = C
    n0 = 0
    while rem > 0:
        if NT_MODE == "aligned":
            if rem > NSPLIT + NMIN // 2:
                sz = NSPLIT
            elif rem > NSPLIT:
                sz = (rem // 2 + P - 1) // P * P
            else:
                sz = rem
        else:
            if rem > 2 * NSPLIT:
                sz = NSPLIT
            elif rem > NSPLIT and rem - NMIN <= NSPLIT:
                # Make the LAST tile exactly NMIN wide: its eviction
                # (CAST + DMA) is the post-stream critical path, and any
                # width >= NMIN streams at the same per-column rate.
                sz = rem - NMIN
            elif rem > NSPLIT:
                sz = (rem + 1) // 2
            else:
                sz = rem
        out.append((n0, sz))
        n0 += sz
        rem -= sz
    return out


def _build_program(D: int, C: int, mm_dt_name: str, out_dt_name: str):
    import concourse.bacc as bacc
    import concourse.mybir as mybir
    import concourse.tile as tile

    mm_dt = getattr(mybir.dt, mm_dt_name)
    out_dt = getattr(mybir.dt, out_dt_name)
    KT = D // P  # k tiles (contraction)
    MT = D // P  # m tiles (output features)
    KC = KT // PASSES  # k tiles per chunk
    n_tiles = _n_tiles(C)
    NT = len(n_tiles)

    nc = bacc.Bacc(None, target_bir_lowering=False)
    xt = nc.declare_dram_parameter("xt", [NT, P, KT * NSPLIT], mm_dt, isOutput=False)
    w = nc.declare_dram_parameter("w", [P, KT * D], mm_dt, isOutput=False)
    yt = nc.declare_dram_parameter("yt", [MT, P, C], out_dt, isOutput=True)

    with tile.TileContext(nc) as tc:
        with (
            tc.tile_pool(name="wpool", bufs=PASSES + 1) as wpool,
            tc.tile_pool(name="xpool", bufs=PASSES) as xpool,
            tc.tile_pool(name="psum", bufs=8, space="PSUM") as psum_pool,
            tc.tile_pool(name="opool", bufs=4) as opool,
            tc.tile_pool(name="warm", bufs=1) as warmpool,
        ):
            if WARMUP_MM:
                # Keep the PE busy from preamble-end so the HAM clock gate is
                # 8/8 when the real matmuls start.  128-wide for fine-grained
                # overrun; the operands are uninitialized SBUF (the product
                # lands in PSUM tiles nothing reads, and every real
                # accumulation group opens with start=True).
                wt = warmpool.tile([P, P], mybir.dt.bfloat16, tag="warm_w")
                if WARM_INIT:
                    nc.vector.memset(wt[:], 0.0)
                for i in range(WARMUP_MM):
                    wp = psum_pool.tile([P, NSPLIT], mybir.dt.float32, tag="ps")
                    nc.tensor.matmul(
                        wp[:, :P], lhsT=wt[:], rhs=wt[:], start=True, stop=True
                    )

            # --- input DMA triggers ---
            # Input rides the two HWDGE rings (sync/SP and scalar/Activation)
            # with per-ring FIFO in consumption order.  Every transfer is
            # shaped for 2KB-contiguous rows: 1KB rows (e.g. a [P,512] slice)
            # halve the DMA packet size and the early ring rate with it
            # (~190GB/s vs ~320GB/s measured), which is exactly the window
            # where the stream is fed.  So W chunks stay [P, KC, D] (2KB
            # rows) and x0 is chunked as [P, 2, 512] (2KB rows, 4 chunks).
            #   sync:   W c0..c5, then x1, x2 (bulk BEHIND all its W)
            #   scalar: x0 q0..q3, then W c6, c7 (consumed last among j=0)
            n0_first, nsz_first = n_tiles[0]
            XKC = 2  # k-tiles per x0 chunk (2KB rows)

            w_sb = []
            w_late = {6, 7}
            for c in range(PASSES):
                if c in w_late:
                    w_sb.append(None)
                    continue
                tw = wpool.tile([P, KC, D], mm_dt, tag="w_sb")
                nc.sync.dma_start(
                    tw[:].rearrange("p k d -> p (k d)"),
                    w[:, c * KC * D : (c + 1) * KC * D],
                )
                w_sb.append(tw)
            x0_sb = []
            for q in range(KT // XKC):
                tx = xpool.tile([P, XKC, nsz_first], mm_dt, tag="x0_sb")
                nc.scalar.dma_start(
                    tx[:].rearrange("p k n -> p (k n)"),
                    xt[0, :, q * XKC * nsz_first : (q + 1) * XKC * nsz_first],
                )
                x0_sb.append(tx)
            for c in sorted(w_late):
                tw = wpool.tile([P, KC, D], mm_dt, tag="w_sb")
                nc.scalar.dma_start(
                    tw[:].rearrange("p k d -> p (k d)"),
                    w[:, c * KC * D : (c + 1) * KC * D],
                )
                w_sb[c] = tw
            # j>=1 x tiles ride the sync HWDGE ring BEHIND every W chunk
            # (per-ring FIFO), so their bulk can never steal engine bandwidth
            # from still-pending W chunks.  On gpsimd's software ring they
            # starved both HWDGE rings outright (v3); on the scalar ring they
            # ran ahead of need and starved W c4..c7 (v4).
            x_sb = [None]
            for j, (n0, nsz) in enumerate(n_tiles):
                if j == 0:
                    continue
                t = xpool.tile([P, KT, nsz], mm_dt, tag="x_sb")
                nc.sync.dma_start(
                    t[:].rearrange("p k n -> p (k n)"),
                    xt[j, :, : KT * nsz],
                )
                x_sb.append(t)

            def mm(ps, mi, j, k, nsz, start, stop):
                if j == 0:
                    rhs = x0_sb[k // XKC][:, k % XKC, :nsz]
                else:
                    rhs = x_sb[j][:, k, :nsz]
                nc.tensor.matmul(
                    ps[:, :nsz],
                    lhsT=w_sb[k // KC][:, k % KC, mi * P : (mi + 1) * P],
                    rhs=rhs,
                    start=start,
                    stop=stop,
                )

            evict_n = [0]

            def evict(ps, mi, n0, nsz):
                ot = opool.tile([P, NSPLIT], out_dt, tag="ot")
                nc.vector.tensor_copy(ot[:, :nsz], ps[:, :nsz])
                evict_n[0] += 1
                if OUT_Q == "alt":
                    q = nc.sync if evict_n[0] % 2 == 1 else nc.scalar
                else:
                    q = nc.sync
                q.dma_start(yt[mi, :, n0 : n0 + nsz], ot[:, :nsz])

            # First n-tile: PASSES k-passes across all m, tracking the
            # arriving W/X0 chunks; evict on the last pass.
            ps0 = []
            for c in range(PASSES):
                for mi in range(MT):
                    if c == 0:
                        ps = psum_pool.tile([P, NSPLIT], mybir.dt.float32, tag="ps")
                        ps0.append(ps)
                    ps = ps0[mi]
                    for k in range(c * KC, (c + 1) * KC):
                        mm(
                            ps,
                            mi,
                            0,
                            k,
                            nsz_first,
                            start=(k == 0),
                            stop=(k == KT - 1),
                        )
                    if c == PASSES - 1:
                        evict(ps, mi, n0_first, nsz_first)
            # Remaining n-tiles: fused k loop.
            for j, (n0, nsz) in enumerate(n_tiles):
                if j == 0:
                    continue
                for mi in range(MT):
                    ps = psum_pool.tile([P, NSPLIT], mybir.dt.float32, tag="ps")
                    for k in range(KT):
                        mm(ps, mi, j, k, nsz, start=(k == 0), stop=(k == KT - 1))
                    evict(ps, mi, n0, nsz)
            if WARMUP_MM and TAIL_MM:
                tp = psum_pool.tile([P, NSPLIT], mybir.dt.float32, tag="ps")
                for i in range(TAIL_MM):
                    nc.tensor.matmul(
                        tp[:, :P], lhsT=wt[:], rhs=wt[:], start=True, stop=True
                    )
    nc.compile()
    return nc


def kernel(hidden: np.ndarray, gate_logits: np.ndarray, W: np.ndarray) -> np.ndarray:
    from concourse.bass_utils import run_bass_kernel_spmd

    hidden = np.asarray(hidden)
    gate_logits = np.asarray(gate_logits)
    W = np.asarray(W)
    B, S, D = hidden.shape
    T, E = gate_logits.shape
    assert E == N_CORES
    x = np.ascontiguousarray(hidden.reshape(T, D).astype(np.float32))

    # --- routing on host (fp32, matches reference softmax/top-2) ---
    g = gate_logits.astype(np.float32)
    m = g.max(axis=-1, keepdims=True)
    p = np.exp(g - m)
    p /= p.sum(axis=-1, keepdims=True)
    top2 = np.argpartition(-p, 1, axis=-1)[:, :2]

    routed = [np.nonzero((top2 == e).any(axis=1))[0] for e in range(E)]
    counts = np.array([len(r) for r in routed])
    if C_MODE == "exact":
        C = max(NMIN, int(counts.max()))
    else:
        C = max(NMIN, int(-(-counts.max() // P)) * P)

    mm_np = _np_dt(MM_DT)
    KT = D // P
    n_tiles = _n_tiles(C)
    NT = len(n_tiles)

    in_maps = []
    for e in range(E):
        idx = routed[e]
        scale = p[idx, e].astype(np.float32)
        xe = x[idx] * scale[:, None]  # [cnt, D]
        xt_full = np.zeros((D, C), dtype=mm_np)
        xt_full[:, : len(idx)] = xe.T.astype(mm_np)
        # [D, C] -> [KT, P, C] -> per-n-tile [NT, P, KT, nsz] contiguous
        xk = xt_full.reshape(KT, P, C)
        xt_dram = np.zeros((NT, P, KT * NSPLIT), dtype=mm_np)
        for j, (n0, nsz) in enumerate(n_tiles):
            blk = xk[:, :, n0 : n0 + nsz].transpose(1, 0, 2)  # [P, KT, nsz]
            xt_dram[j, :, : KT * nsz] = blk.reshape(P, KT * nsz)
        w_full = (
            W[e].astype(mm_np).reshape(KT, P, D).transpose(1, 0, 2).reshape(P, KT * D)
        )
        in_maps.append(
            {
                "xt": np.ascontiguousarray(xt_dram),
                "w": np.ascontiguousarray(w_full),
            }
        )

    key = (D, C, MM_DT, OUT_DT, WARMUP_MM, TAIL_MM, OUT_Q, NT_MODE, WARM_INIT)
    if key not in _prog_cache:
        _prog_cache[key] = _build_program(D, C, MM_DT, OUT_DT)
    nc = _prog_cache[key]

    res = run_bass_kernel_spmd(nc, in_maps, core_ids=list(range(N_CORES)))

    # --- combine on host ---
    out = np.zeros((T, D), dtype=np.float32)
    for e in range(E):
        idx = routed[e]
        ye_t = res.results[e]["yt"].reshape(D, C)  # Y^T
        out[idx] += ye_t[:, : len(idx)].T.astype(np.float32)
    return out.reshape(B, S, D)


# revision 23
# speedup vs baseline: 1.0201x; 1.0201x over previous
"""MoE dispatcher kernel for Trainium2 (8 NeuronCores, expert-parallel).

Contract: kernel(**inputs) takes FULL inputs and returns the FULL output.

Strategy (expert-parallel, matches the sharding hint):
  - host: softmax(gate_logits) -> top-2 -> combine weights per (token, expert)
  - host "all-to-all dispatch": for expert e, gather its routed tokens,
    pre-scale rows by the combine weight (w * (x @ W) == (w*x) @ W), pad to a
    common capacity C, transpose to [D, C] so the device streams tokens along
    the free dim.  One expert per core.
  - device (per core): Y^T[D,C] = W[e]^T @ X^T via PE array, tiled
    [128 x <=512] PSUM accumulation over K=D.
  - host "all-to-all combine": scatter-add each expert's Y rows back to the
    token axis (plain add; weights were folded into x).

Measured structure (ntff forensics, 49.0us baseline):
  - exec_time_ns = (last teardown instruction end) - (first framework MEMSET).
    The ~5.9us runtime preamble before the memset is NOT counted; the ~7.2us
    teardown (all 254 semaphores cleared one inst each, engine-partitioned,
    Tensor's 53-clear chain at 115ns each is critical) IS counted and is
    runtime-fixed.
  - The teardown chain rate is clock-gate independent (115ns/clear at both
    K=4/8 and 8/8), so tail dummy matmuls only push the end barrier out;
    TAIL_MM now defaults 0.
  - Head: engines exit the framework preamble ~7.0-7.2us.  DMA triggers take
    ~0.65us of descriptor-gen each and the ring adds ~1.3us latency, so the
    first input bytes land ~8.7us.  v1 serialized all W chunks on one queue
    and memset the warm tile first: first real MM at ~12.4us, HAM clock warm
    only at ~13.9us.
  - This version (measured 48.7us max-core vs 49.0 v1): warmup matmuls are
    128-wide (107ns cold granularity) into a dead PSUM tile, starting right
    after a ~0.1us memset at ~7.0us; 40 of them bridge past the worst-case
    first-data arrival so no core's PE ever idles >3.4us (which would HAM
    re-throttle its whole stream).  W (k=0 as two halves, then c1..c7, then
    the j>=1 x tiles) rides sync's HWDGE ring; the x0 chunks ride scalar's;
    descriptor generation overlaps and per-ring FIFO preserves consumption
    order.  First real MM ~10.2-11.8us (per-core DMA arrival variance),
    stream gapless and warm.  Failed variants, for the record: bulk x tiles
    on gpsimd's SWDGE starve both HWDGE rings; bulk x tiles early on the
    scalar ring starve late W chunks; tail dummy matmuls and a split final
    evict DMA both lengthen the (fully counted) teardown; 2-k-wide x0 chunks
    delay the first pass more than their bigger DMA packets help.

DRAM layouts are host-permuted so every DMA is fully contiguous per
partition:
  w   [P, KT*D]            w[p, k*D + d] = W[e][k*128 + p, d]
  xt  [NT, P, KT*NSPLIT]   xt[j, p, k*nsz_j + n] = X^T[k*128 + p, n0_j + n]
  yt  [MT, P, C]           yt[m, p, n]   = Y^T[m*128 + p, n]
"""

import os

import numpy as np

N_CORES = 8
P = 128
NSPLIT = 512  # max moving-operand / PSUM-bank free dim (fp32)
NMIN = 256  # keep moving tiles >=256 wide (per-MM floor ~107ns below that)
PASSES = 8  # k-dim chunks for W / first-n-tile pipelining

MM_DT = os.environ.get("BASS_MOE_DT", "bfloat16")
OUT_DT = os.environ.get("BASS_MOE_OUT_DT", "bfloat16")
# 128-wide warmup matmuls bridging preamble-end (~7.0us) past the WORST-CASE
# first-data arrival (~12.5us on an unlucky core).  Cores whose data lands
# early pay ~56ns per excess warmup; a core whose data is late but whose
# warmups ended early idles >3.4us and HAM re-throttles its whole stream
# (v4 core6: +1.7us).  The graded metric is the max core, so over-provision.
WARMUP_MM = int(os.environ.get("BASS_MOE_WARMUP", "40"))
# dummy matmuls after the last real one (teardown chain is clock-independent,
# so these only delay the end barrier; 0 unless measurement says otherwise)
TAIL_MM = int(os.environ.get("BASS_MOE_TAIL", "0"))
OUT_Q = os.environ.get("BASS_MOE_OUT_Q", "alt")
NT_MODE = os.environ.get("BASS_MOE_NT", "even")
C_MODE = os.environ.get("BASS_MOE_C", "exact")
# the warm tile is only [P, 128] now, so its memset is ~0.1us (the tile
# scheduler requires every read tile to have a writer, so 0 is not allowed)
WARM_INIT = int(os.environ.get("BASS_MOE_WARM_INIT", "1"))

_prog_cache: dict = {}


def _np_dt(name):
    if name == "bfloat16":
        import ml_dtypes

        return ml_dtypes.bfloat16
    return np.float32


def _n_tiles(C):
    """Split C into tiles of at most NSPLIT (remainder split evenly)."""
    out = []
    rem = C
    n0 = 0
    while rem > 0:
        if NT_MODE == "aligned":
            if rem > NSPLIT + NMIN // 2:
                sz = NSPLIT
            elif rem > NSPLIT:
                sz = (rem // 2 + P - 1) // P * P
            else:
                sz = rem
        else:
            if rem > 2 * NSPLIT:
                sz = NSPLIT
            elif rem > NSPLIT:
                sz = (rem + 1) // 2
            else:
                sz = rem
        out.append((n0, sz))
        n0 += sz
        rem -= sz
    return out


def _build_program(D: int, C: int, mm_dt_name: str, out_dt_name: str):
    import concourse.bacc as bacc
    import concourse.mybir as mybir
    import concourse.tile as tile

    mm_dt = getattr(mybir.dt, mm_dt_name)
    out_dt = getattr(mybir.dt, out_dt_name)
    KT = D // P  # k tiles (contraction)
    MT = D // P  # m tiles (output features)
    KC = KT // PASSES  # k tiles per chunk
    n_tiles = _n_tiles(C)
    NT = len(n_tiles)

    nc = bacc.Bacc(None, target_bir_lowering=False)
    xt = nc.declare_dram_parameter("xt", [NT, P, KT * NSPLIT], mm_dt, isOutput=False)
    w = nc.declare_dram_parameter("w", [P, KT * D], mm_dt, isOutput=False)
    yt = nc.declare_dram_parameter("yt", [MT, P, C], out_dt, isOutput=True)

    with tile.TileContext(nc) as tc:
        with (
            tc.tile_pool(name="wpool", bufs=PASSES + 1) as wpool,
            tc.tile_pool(name="xpool", bufs=PASSES) as xpool,
            tc.tile_pool(name="psum", bufs=8, space="PSUM") as psum_pool,
            tc.tile_pool(name="opool", bufs=4) as opool,
            tc.tile_pool(name="warm", bufs=1) as warmpool,
        ):
            if WARMUP_MM:
                # Keep the PE busy from preamble-end so the HAM clock gate is
                # 8/8 when the real matmuls start.  128-wide for fine-grained
                # overrun; the operands are uninitialized SBUF (the product
                # lands in PSUM tiles nothing reads, and every real
                # accumulation group opens with start=True).
                wt = warmpool.tile([P, P], mybir.dt.bfloat16, tag="warm_w")
                if WARM_INIT:
                    nc.vector.memset(wt[:], 0.0)
                for i in range(WARMUP_MM):
                    wp = psum_pool.tile([P, NSPLIT], mybir.dt.float32, tag="ps")
                    nc.tensor.matmul(
                        wp[:, :P], lhsT=wt[:], rhs=wt[:], start=True, stop=True
                    )

            # --- input DMA triggers ---
            # Input rides the two HWDGE rings so each k-chunk pair (W-c on
            # sync's ring, x0-c on scalar's ring) is delivered by an
            # independent hardware path with parallel descriptor generation;
            # per-ring FIFO keeps chunks in consumption order.
            n0_first, nsz_first = n_tiles[0]
            half = D // 2

            w0a = wpool.tile([P, KC, half], mm_dt, tag="w0a")
            nc.sync.dma_start(w0a[:].rearrange("p k d -> p (k d)"), w[:, 0:half])
            x0_sb = []
            tx = xpool.tile([P, KC, nsz_first], mm_dt, tag="x0_sb")
            nc.scalar.dma_start(
                tx[:].rearrange("p k n -> p (k n)"), xt[0, :, 0 : KC * nsz_first]
            )
            x0_sb.append(tx)
            w0b = wpool.tile([P, KC, half], mm_dt, tag="w0b")
            nc.sync.dma_start(w0b[:].rearrange("p k d -> p (k d)"), w[:, half:D])

            w_sb = [None]
            for c in range(1, PASSES):
                tw = wpool.tile([P, KC, D], mm_dt, tag="w_sb")
                nc.sync.dma_start(
                    tw[:].rearrange("p k d -> p (k d)"),
                    w[:, c * KC * D : (c + 1) * KC * D],
                )
                w_sb.append(tw)
            for c in range(1, PASSES):
                tx = xpool.tile([P, KC, nsz_first], mm_dt, tag="x0_sb")
                nc.scalar.dma_start(
                    tx[:].rearrange("p k n -> p (k n)"),
                    xt[0, :, c * KC * nsz_first : (c + 1) * KC * nsz_first],
                )
                x0_sb.append(tx)
            # j>=1 x tiles ride the sync HWDGE ring BEHIND every W chunk
            # (per-ring FIFO), so their bulk can never steal engine bandwidth
            # from still-pending W chunks.  On gpsimd's software ring they
            # starved both HWDGE rings outright (v3); on the scalar ring they
            # ran ahead of need and starved W c4..c7 (v4).
            x_sb = [None]
            for j, (n0, nsz) in enumerate(n_tiles):
                if j == 0:
                    continue
                t = xpool.tile([P, KT, nsz], mm_dt, tag="x_sb")
                nc.sync.dma_start(
                    t[:].rearrange("p k n -> p (k n)"),
                    xt[j, :, : KT * nsz],
                )
                x_sb.append(t)

            def w_slice(k, mi):
                if k // KC == 0:
                    m0 = mi * P
                    if m0 < half:
                        return w0a[:, k % KC, m0 : m0 + P]
                    return w0b[:, k % KC, m0 - half : m0 - half + P]
                return w_sb[k // KC][:, k % KC, mi * P : (mi + 1) * P]

            def mm(ps, mi, j, k, nsz, start, stop):
                if j == 0:
                    rhs = x0_sb[k // KC][:, k % KC, :nsz]
                else:
                    rhs = x_sb[j][:, k, :nsz]
                nc.tensor.matmul(
                    ps[:, :nsz],
                    lhsT=w_slice(k, mi),
                    rhs=rhs,
                    start=start,
                    stop=stop,
                )

            evict_n = [0]

            def evict(ps, mi, n0, nsz):
                ot = opool.tile([P, NSPLIT], out_dt, tag="ot")
                nc.vector.tensor_copy(ot[:, :nsz], ps[:, :nsz])
                evict_n[0] += 1
                if OUT_Q == "alt":
                    q = nc.sync if evict_n[0] % 2 == 1 else nc.scalar
                else:
                    q = nc.sync
                q.dma_start(yt[mi, :, n0 : n0 + nsz], ot[:, :nsz])

            # First n-tile: PASSES k-passes across all m, tracking the
            # arriving W/X0 chunks; evict on the last pass.
            ps0 = []
            for c in range(PASSES):
                for mi in range(MT):
                    if c == 0:
                        ps = psum_pool.tile([P, NSPLIT], mybir.dt.float32, tag="ps")
                        ps0.append(ps)
                    ps = ps0[mi]
                    for k in range(c * KC, (c + 1) * KC):
                        mm(
                            ps,
                            mi,
                            0,
                            k,
                            nsz_first,
                            start=(k == 0),
                            stop=(k == KT - 1),
                        )
                    if c == PASSES - 1:
                        evict(ps, mi, n0_first, nsz_first)
            # Remaining n-tiles: fused k loop.
            for j, (n0, nsz) in enumerate(n_tiles):
                if j == 0:
                    continue
                for mi in range(MT):
                    ps = psum_pool.tile([P, NSPLIT], mybir.dt.float32, tag="ps")
                    for k in range(KT):
                        mm(ps, mi, j, k, nsz, start=(k == 0), stop=(k == KT - 1))
                    evict(ps, mi, n0, nsz)
            if WARMUP_MM and TAIL_MM:
                tp = psum_pool.tile([P, NSPLIT], mybir.dt.float32, tag="ps")
                for i in range(TAIL_MM):
                    nc.tensor.matmul(
                        tp[:, :P], lhsT=wt[:], rhs=wt[:], start=True, stop=True
                    )
    nc.compile()
    return nc


def kernel(hidden: np.ndarray, gate_logits: np.ndarray, W: np.ndarray) -> np.ndarray:
    from concourse.bass_utils import run_bass_kernel_spmd

    hidden = np.asarray(hidden)
    gate_logits = np.asarray(gate_logits)
    W = np.asarray(W)
    B, S, D = hidden.shape
    T, E = gate_logits.shape
    assert E == N_CORES
    x = np.ascontiguousarray(hidden.reshape(T, D).astype(np.float32))

    # --- routing on host (fp32, matches reference softmax/top-2) ---
    g = gate_logits.astype(np.float32)
    m = g.max(axis=-1, keepdims=True)
    p = np.exp(g - m)
    p /= p.sum(axis=-1, keepdims=True)
    top2 = np.argpartition(-p, 1, axis=-1)[:, :2]

    routed = [np.nonzero((top2 == e).any(axis=1))[0] for e in range(E)]
    counts = np.array([len(r) for r in routed])
    if C_MODE == "exact":
        C = max(NMIN, int(counts.max()))
    else:
        C = max(NMIN, int(-(-counts.max() // P)) * P)

    mm_np = _np_dt(MM_DT)
    KT = D // P
    n_tiles = _n_tiles(C)
    NT = len(n_tiles)

    in_maps = []
    for e in range(E):
        idx = routed[e]
        scale = p[idx, e].astype(np.float32)
        xe = x[idx] * scale[:, None]  # [cnt, D]
        xt_full = np.zeros((D, C), dtype=mm_np)
        xt_full[:, : len(idx)] = xe.T.astype(mm_np)
        # [D, C] -> [KT, P, C] -> per-n-tile [NT, P, KT, nsz] contiguous
        xk = xt_full.reshape(KT, P, C)
        xt_dram = np.zeros((NT, P, KT * NSPLIT), dtype=mm_np)
        for j, (n0, nsz) in enumerate(n_tiles):
            blk = xk[:, :, n0 : n0 + nsz].transpose(1, 0, 2)  # [P, KT, nsz]
            xt_dram[j, :, : KT * nsz] = blk.reshape(P, KT * nsz)
        w_full = (
            W[e].astype(mm_np).reshape(KT, P, D).transpose(1, 0, 2).reshape(P, KT * D)
        )
        in_maps.append(
            {
                "xt": np.ascontiguousarray(xt_dram),
                "w": np.ascontiguousarray(w_full),
            }
        )

    key = (D, C, MM_DT, OUT_DT, WARMUP_MM, TAIL_MM, OUT_Q, NT_MODE, WARM_INIT)
    if key not in _prog_cache:
        _prog_cache[key] = _build_program(D, C, MM_DT, OUT_DT)
    nc = _prog_cache[key]

    res = run_bass_kernel_spmd(nc, in_maps, core_ids=list(range(N_CORES)))

    # --- combine on host ---
    out = np.zeros((T, D), dtype=np.float32)
    for e in range(E):
        idx = routed[e]
        ye_t = res.results[e]["yt"].reshape(D, C)  # Y^T
        out[idx] += ye_t[:, : len(idx)].T.astype(np.float32)
    return out.reshape(B, S, D)


# revision 26
# speedup vs baseline: 1.0254x; 1.0052x over previous
"""MoE dispatcher kernel for Trainium2 (8 NeuronCores, expert-parallel).

Contract: kernel(**inputs) takes FULL inputs and returns the FULL output.

Strategy (expert-parallel, matches the sharding hint):
  - host: softmax(gate_logits) -> top-2 -> combine weights per (token, expert)
  - host "all-to-all dispatch": for expert e, gather its routed tokens,
    pre-scale rows by the combine weight (w * (x @ W) == (w*x) @ W), pad to a
    common capacity C, transpose to [D, C] so the device streams tokens along
    the free dim.  One expert per core.
  - device (per core): Y^T[D,C] = W[e]^T @ X^T via PE array, tiled
    [128 x <=512] PSUM accumulation over K=D.
  - host "all-to-all combine": scatter-add each expert's Y rows back to the
    token axis (plain add; weights were folded into x).

Measured structure (ntff forensics, 49.0us baseline):
  - exec_time_ns = (last teardown instruction end) - (first framework MEMSET).
    The ~5.9us runtime preamble before the memset is NOT counted; the ~7.2us
    teardown (all 254 semaphores cleared one inst each, engine-partitioned,
    Tensor's 53-clear chain at 115ns each is critical) IS counted and is
    runtime-fixed.
  - The teardown chain rate is clock-gate independent (115ns/clear at both
    K=4/8 and 8/8), so tail dummy matmuls only push the end barrier out;
    TAIL_MM now defaults 0.
  - Head: engines exit the framework preamble ~7.0-7.2us.  DMA triggers take
    ~0.65us of descriptor-gen each and the ring adds ~1.3us latency, so the
    first input bytes land ~8.7us.  v1 serialized all W chunks on one queue
    and memset the warm tile first: first real MM at ~12.4us, HAM clock warm
    only at ~13.9us.
  - This version (measured 48.7us max-core vs 49.0 v1): warmup matmuls are
    128-wide (107ns cold granularity) into a dead PSUM tile, starting right
    after a ~0.1us memset at ~7.0us; 40 of them bridge past the worst-case
    first-data arrival so no core's PE ever idles >3.4us (which would HAM
    re-throttle its whole stream).  W (k=0 as two halves, then c1..c7, then
    the j>=1 x tiles) rides sync's HWDGE ring; the x0 chunks ride scalar's;
    descriptor generation overlaps and per-ring FIFO preserves consumption
    order.  First real MM ~10.2-11.8us (per-core DMA arrival variance),
    stream gapless and warm.  Failed variants, for the record: bulk x tiles
    on gpsimd's SWDGE starve both HWDGE rings; bulk x tiles early on the
    scalar ring starve late W chunks; tail dummy matmuls and a split final
    evict DMA both lengthen the (fully counted) teardown; 2-k-wide x0 chunks
    delay the first pass more than their bigger DMA packets help.

DRAM layouts are host-permuted so every DMA is fully contiguous per
partition:
  w   [P, KT*D]            w[p, k*D + d] = W[e][k*128 + p, d]
  xt  [NT, P, KT*NSPLIT]   xt[j, p, k*nsz_j + n] = X^T[k*128 + p, n0_j + n]
  yt  [MT, P, C]           yt[m, p, n]   = Y^T[m*128 + p, n]
"""

import os

import numpy as np

N_CORES = 8
P = 128
NSPLIT = 512  # max moving-operand / PSUM-bank free dim (fp32)
NMIN = 256  # keep moving tiles >=256 wide (per-MM floor ~107ns below that)
PASSES = 8  # k-dim chunks for W / first-n-tile pipelining

MM_DT = os.environ.get("BASS_MOE_DT", "bfloat16")
OUT_DT = os.environ.get("BASS_MOE_OUT_DT", "bfloat16")
# 128-wide warmup matmuls bridging preamble-end (~7.0us) past the WORST-CASE
# first-data arrival (~12.5us on an unlucky core).  Cores whose data lands
# early pay ~56ns per excess warmup; a core whose data is late but whose
# warmups ended early idles >3.4us and HAM re-throttles its whole stream
# (v4 core6: +1.7us).  The graded metric is the max core, so over-provision.
WARMUP_MM = int(os.environ.get("BASS_MOE_WARMUP", "40"))
# dummy matmuls after the last real one (teardown chain is clock-independent,
# so these only delay the end barrier; 0 unless measurement says otherwise)
TAIL_MM = int(os.environ.get("BASS_MOE_TAIL", "0"))
OUT_Q = os.environ.get("BASS_MOE_OUT_Q", "alt")
NT_MODE = os.environ.get("BASS_MOE_NT", "even")
C_MODE = os.environ.get("BASS_MOE_C", "exact")
# the warm tile is only [P, 128] now, so its memset is ~0.1us (the tile
# scheduler requires every read tile to have a writer, so 0 is not allowed)
WARM_INIT = int(os.environ.get("BASS_MOE_WARM_INIT", "1"))

_prog_cache: dict = {}


def _np_dt(name):
    if name == "bfloat16":
        import ml_dtypes

        return ml_dtypes.bfloat16
    return np.float32


def _n_tiles(C):
    """Split C into tiles of at most NSPLIT (remainder split evenly)."""
    out = []
    rem = C
    n0 = 0
    while rem > 0:
        if NT_MODE == "aligned":
            if rem > NSPLIT + NMIN // 2:
                sz = NSPLIT
            elif rem > NSPLIT:
                sz = (rem // 2 + P - 1) // P * P
            else:
                sz = rem
        else:
            if rem > 2 * NSPLIT:
                sz = NSPLIT
            elif rem > NSPLIT:
                sz = (rem + 1) // 2
            else:
                sz = rem
        out.append((n0, sz))
        n0 += sz
        rem -= sz
    return out


def _build_program(D: int, C: int, mm_dt_name: str, out_dt_name: str):
    import concourse.bacc as bacc
    import concourse.mybir as mybir
    import concourse.tile as tile

    mm_dt = getattr(mybir.dt, mm_dt_name)
    out_dt = getattr(mybir.dt, out_dt_name)
    KT = D // P  # k tiles (contraction)
    MT = D // P  # m tiles (output features)
    KC = KT // PASSES  # k tiles per chunk
    n_tiles = _n_tiles(C)
    NT = len(n_tiles)

    nc = bacc.Bacc(None, target_bir_lowering=False)
    xt = nc.declare_dram_parameter("xt", [NT, P, KT * NSPLIT], mm_dt, isOutput=False)
    w = nc.declare_dram_parameter("w", [P, KT * D], mm_dt, isOutput=False)
    yt = nc.declare_dram_parameter("yt", [MT, P, C], out_dt, isOutput=True)

    with tile.TileContext(nc) as tc:
        with (
            tc.tile_pool(name="wpool", bufs=PASSES + 1) as wpool,
            tc.tile_pool(name="xpool", bufs=PASSES) as xpool,
            tc.tile_pool(name="psum", bufs=8, space="PSUM") as psum_pool,
            tc.tile_pool(name="opool", bufs=4) as opool,
            tc.tile_pool(name="warm", bufs=1) as warmpool,
        ):
            if WARMUP_MM:
                # Keep the PE busy from preamble-end so the HAM clock gate is
                # 8/8 when the real matmuls start.  128-wide for fine-grained
                # overrun; the operands are uninitialized SBUF (the product
                # lands in PSUM tiles nothing reads, and every real
                # accumulation group opens with start=True).
                wt = warmpool.tile([P, P], mybir.dt.bfloat16, tag="warm_w")
                if WARM_INIT:
                    nc.vector.memset(wt[:], 0.0)
                for i in range(WARMUP_MM):
                    wp = psum_pool.tile([P, NSPLIT], mybir.dt.float32, tag="ps")
                    nc.tensor.matmul(
                        wp[:, :P], lhsT=wt[:], rhs=wt[:], start=True, stop=True
                    )

            # --- input DMA triggers ---
            # Input rides the two HWDGE rings so each k-chunk pair (W-c on
            # sync's ring, x0-c on scalar's ring) is delivered by an
            # independent hardware path with parallel descriptor generation;
            # per-ring FIFO keeps chunks in consumption order.
            n0_first, nsz_first = n_tiles[0]
            half = D // 2

            w0a = wpool.tile([P, KC, half], mm_dt, tag="w0a")
            nc.sync.dma_start(w0a[:].rearrange("p k d -> p (k d)"), w[:, 0:half])
            x0_sb = []
            tx = xpool.tile([P, KC, nsz_first], mm_dt, tag="x0_sb")
            nc.scalar.dma_start(
                tx[:].rearrange("p k n -> p (k n)"), xt[0, :, 0 : KC * nsz_first]
            )
            x0_sb.append((tx, 0, 1))
            w0b = wpool.tile([P, KC, half], mm_dt, tag="w0b")
            nc.sync.dma_start(w0b[:].rearrange("p k d -> p (k d)"), w[:, half:D])

            w_sb = [None]
            for c in range(1, PASSES):
                tw = wpool.tile([P, KC, D], mm_dt, tag="w_sb")
                nc.sync.dma_start(
                    tw[:].rearrange("p k d -> p (k d)"),
                    w[:, c * KC * D : (c + 1) * KC * D],
                )
                w_sb.append(tw)
            # Remaining x0 k-tiles ride in PAIRS: a [P, 2, 512] slice has
            # 2KB-contiguous rows, so its DMA packets are 2KB and move at
            # ~320GB/s instead of the 1KB-packet ~190GB/s.  This frees the
            # scalar ring ~1.6us earlier, exactly the window where late W
            # chunks otherwise gap the stream.  c0 stays a single k so the
            # first matmul's data gate stays 128KB.
            x0_pairs = [(1, 2), (3, 2), (5, 2), (7, 1)]
            for k0, nk in x0_pairs:
                tx = xpool.tile([P, nk, nsz_first], mm_dt, tag="x0_sb")
                nc.scalar.dma_start(
                    tx[:].rearrange("p k n -> p (k n)"),
                    xt[0, :, k0 * nsz_first : (k0 + nk) * nsz_first],
                )
                x0_sb.append((tx, k0, nk))
            # j>=1 x tiles ride the sync HWDGE ring BEHIND every W chunk
            # (per-ring FIFO), so their bulk can never steal engine bandwidth
            # from still-pending W chunks.  On gpsimd's software ring they
            # starved both HWDGE rings outright (v3); on the scalar ring they
            # ran ahead of need and starved W c4..c7 (v4).
            x_sb = [None]
            for j, (n0, nsz) in enumerate(n_tiles):
                if j == 0:
                    continue
                t = xpool.tile([P, KT, nsz], mm_dt, tag="x_sb")
                nc.sync.dma_start(
                    t[:].rearrange("p k n -> p (k n)"),
                    xt[j, :, : KT * nsz],
                )
                x_sb.append(t)

            def w_slice(k, mi):
                if k // KC == 0:
                    m0 = mi * P
                    if m0 < half:
                        return w0a[:, k % KC, m0 : m0 + P]
                    return w0b[:, k % KC, m0 - half : m0 - half + P]
                return w_sb[k // KC][:, k % KC, mi * P : (mi + 1) * P]

            def x0_rhs(k, nsz):
                for t, k0, nk in x0_sb:
                    if k0 <= k < k0 + nk:
                        return t[:, k - k0, :nsz]
                raise AssertionError(k)

            def mm(ps, mi, j, k, nsz, start, stop):
                if j == 0:
                    rhs = x0_rhs(k, nsz)
                else:
                    rhs = x_sb[j][:, k, :nsz]
                nc.tensor.matmul(
                    ps[:, :nsz],
                    lhsT=w_slice(k, mi),
                    rhs=rhs,
                    start=start,
                    stop=stop,
                )

            evict_n = [0]

            def evict(ps, mi, n0, nsz):
                ot = opool.tile([P, NSPLIT], out_dt, tag="ot")
                nc.vector.tensor_copy(ot[:, :nsz], ps[:, :nsz])
                evict_n[0] += 1
                if OUT_Q == "alt":
                    q = nc.sync if evict_n[0] % 2 == 1 else nc.scalar
                else:
                    q = nc.sync
                q.dma_start(yt[mi, :, n0 : n0 + nsz], ot[:, :nsz])

            # First n-tile: PASSES k-passes across all m, tracking the
            # arriving W/X0 chunks; evict on the last pass.
            ps0 = []
            for c in range(PASSES):
                for mi in range(MT):
                    if c == 0:
                        ps = psum_pool.tile([P, NSPLIT], mybir.dt.float32, tag="ps")
                        ps0.append(ps)
                    ps = ps0[mi]
                    for k in range(c * KC, (c + 1) * KC):
                        mm(
                            ps,
                            mi,
                            0,
                            k,
                            nsz_first,
                            start=(k == 0),
                            stop=(k == KT - 1),
                        )
                    if c == PASSES - 1:
                        evict(ps, mi, n0_first, nsz_first)
            # Remaining n-tiles: fused k loop.
            for j, (n0, nsz) in enumerate(n_tiles):
                if j == 0:
                    continue
                for mi in range(MT):
                    ps = psum_pool.tile([P, NSPLIT], mybir.dt.float32, tag="ps")
                    for k in range(KT):
                        mm(ps, mi, j, k, nsz, start=(k == 0), stop=(k == KT - 1))
                    evict(ps, mi, n0, nsz)
            if WARMUP_MM and TAIL_MM:
                tp = psum_pool.tile([P, NSPLIT], mybir.dt.float32, tag="ps")
                for i in range(TAIL_MM):
                    nc.tensor.matmul(
                        tp[:, :P], lhsT=wt[:], rhs=wt[:], start=True, stop=True
                    )
    nc.compile()
    return nc


def kernel(hidden: np.ndarray, gate_logits: np.ndarray, W: np.ndarray) -> np.ndarray:
    from concourse.bass_utils import run_bass_kernel_spmd

    hidden = np.asarray(hidden)
    gate_logits = np.asarray(gate_logits)
    W = np.asarray(W)
    B, S, D = hidden.shape
    T, E = gate_logits.shape
    assert E == N_CORES
    x = np.ascontiguousarray(hidden.reshape(T, D).astype(np.float32))

    # --- routing on host (fp32, matches reference softmax/top-2) ---
    g = gate_logits.astype(np.float32)
    m = g.max(axis=-1, keepdims=True)
    p = np.exp(g - m)
    p /= p.sum(axis=-1, keepdims=True)
    top2 = np.argpartition(-p, 1, axis=-1)[:, :2]

    routed = [np.nonzero((top2 == e).any(axis=1))[0] for e in range(E)]
    counts = np.array([len(r) for r in routed])
    if C_MODE == "exact":
        C = max(NMIN, int(counts.max()))
    else:
        C = max(NMIN, int(-(-counts.max() // P)) * P)

    mm_np = _np_dt(MM_DT)
    KT = D // P
    n_tiles = _n_tiles(C)
    NT = len(n_tiles)

    in_maps = []
    for e in range(E):
        idx = routed[e]
        scale = p[idx, e].astype(np.float32)
        xe = x[idx] * scale[:, None]  # [cnt, D]
        xt_full = np.zeros((D, C), dtype=mm_np)
        xt_full[:, : len(idx)] = xe.T.astype(mm_np)
        # [D, C] -> [KT, P, C] -> per-n-tile [NT, P, KT, nsz] contiguous
        xk = xt_full.reshape(KT, P, C)
        xt_dram = np.zeros((NT, P, KT * NSPLIT), dtype=mm_np)
        for j, (n0, nsz) in enumerate(n_tiles):
            blk = xk[:, :, n0 : n0 + nsz].transpose(1, 0, 2)  # [P, KT, nsz]
            xt_dram[j, :, : KT * nsz] = blk.reshape(P, KT * nsz)
        w_full = (
            W[e].astype(mm_np).reshape(KT, P, D).transpose(1, 0, 2).reshape(P, KT * D)
        )
        in_maps.append(
            {
                "xt": np.ascontiguousarray(xt_dram),
                "w": np.ascontiguousarray(w_full),
            }
        )

    key = (D, C, MM_DT, OUT_DT, WARMUP_MM, TAIL_MM, OUT_Q, NT_MODE, WARM_INIT)
    if key not in _prog_cache:
        _prog_cache[key] = _build_program(D, C, MM_DT, OUT_DT)
    nc = _prog_cache[key]

    res = run_bass_kernel_spmd(nc, in_maps, core_ids=list(range(N_CORES)))

    # --- combine on host ---
    out = np.zeros((T, D), dtype=np.float32)
    for e in range(E):
        idx = routed[e]
        ye_t = res.results[e]["yt"].reshape(D, C)  # Y^T
        out[idx] += ye_t[:, : len(idx)].T.astype(np.float32)
    return out.reshape(B, S, D)


# revision 27
# speedup vs baseline: 1.0454x; 1.0194x over previous
"""MoE dispatcher kernel for Trainium2 (8 NeuronCores, expert-parallel).

Contract: kernel(**inputs) takes FULL inputs and returns the FULL output.

Strategy (expert-parallel, matches the sharding hint):
  - host: softmax(gate_logits) -> top-2 -> combine weights per (token, expert)
  - host "all-to-all dispatch": for expert e, gather its routed tokens,
    pre-scale rows by the combine weight (w * (x @ W) == (w*x) @ W), pad to a
    common capacity C, transpose to [D, C] so the device streams tokens along
    the free dim.  One expert per core.
  - device (per core): Y^T[D,C] = W[e]^T @ X^T via PE array, tiled
    [128 x <=512] PSUM accumulation over K=D.
  - host "all-to-all combine": scatter-add each expert's Y rows back to the
    token axis (plain add; weights were folded into x).

Measured structure (ntff forensics, 49.0us baseline):
  - exec_time_ns = (last teardown instruction end) - (first framework MEMSET).
    The ~5.9us runtime preamble before the memset is NOT counted; the ~7.2us
    teardown (all 254 semaphores cleared one inst each, engine-partitioned,
    Tensor's 53-clear chain at 115ns each is critical) IS counted and is
    runtime-fixed.
  - The teardown chain rate is clock-gate independent (115ns/clear at both
    K=4/8 and 8/8), so tail dummy matmuls only push the end barrier out;
    TAIL_MM now defaults 0.
  - Head: engines exit the framework preamble ~7.0-7.2us.  DMA triggers take
    ~0.65us of descriptor-gen each and the ring adds ~1.3us latency, so the
    first input bytes land ~8.7us.  v1 serialized all W chunks on one queue
    and memset the warm tile first: first real MM at ~12.4us, HAM clock warm
    only at ~13.9us.
  - This version (measured 48.7us max-core vs 49.0 v1): warmup matmuls are
    128-wide (107ns cold granularity) into a dead PSUM tile, starting right
    after a ~0.1us memset at ~7.0us; 40 of them bridge past the worst-case
    first-data arrival so no core's PE ever idles >3.4us (which would HAM
    re-throttle its whole stream).  W (k=0 as two halves, then c1..c7, then
    the j>=1 x tiles) rides sync's HWDGE ring; the x0 chunks ride scalar's;
    descriptor generation overlaps and per-ring FIFO preserves consumption
    order.  First real MM ~10.2-11.8us (per-core DMA arrival variance),
    stream gapless and warm.  Failed variants, for the record: bulk x tiles
    on gpsimd's SWDGE starve both HWDGE rings; bulk x tiles early on the
    scalar ring starve late W chunks; tail dummy matmuls and a split final
    evict DMA both lengthen the (fully counted) teardown; 2-k-wide x0 chunks
    delay the first pass more than their bigger DMA packets help.

DRAM layouts are host-permuted so every DMA is fully contiguous per
partition:
  w   [P, KT*D]            w[p, k*D + d] = W[e][k*128 + p, d]
  xt  [NT, P, KT*NSPLIT]   xt[j, p, k*nsz_j + n] = X^T[k*128 + p, n0_j + n]
  yt  [MT, P, C]           yt[m, p, n]   = Y^T[m*128 + p, n]
"""

import os

import numpy as np

N_CORES = 8
P = 128
NSPLIT = 512  # max moving-operand / PSUM-bank free dim (fp32)
NMIN = 256  # keep moving tiles >=256 wide (per-MM floor ~107ns below that)
PASSES = 8  # k-dim chunks for W / first-n-tile pipelining

MM_DT = os.environ.get("BASS_MOE_DT", "bfloat16")
OUT_DT = os.environ.get("BASS_MOE_OUT_DT", "bfloat16")
# 128-wide warmup matmuls bridging preamble-end (~7.0us) past the WORST-CASE
# first-data arrival (~12.5us on an unlucky core).  Cores whose data lands
# early pay ~56ns per excess warmup; a core whose data is late but whose
# warmups ended early idles >3.4us and HAM re-throttles its whole stream
# (v4 core6: +1.7us).  The graded metric is the max core, so over-provision.
WARMUP_MM = int(os.environ.get("BASS_MOE_WARMUP", "40"))
# dummy matmuls after the last real one (teardown chain is clock-independent,
# so these only delay the end barrier; 0 unless measurement says otherwise)
TAIL_MM = int(os.environ.get("BASS_MOE_TAIL", "0"))
OUT_Q = os.environ.get("BASS_MOE_OUT_Q", "alt")
NT_MODE = os.environ.get("BASS_MOE_NT", "even")
C_MODE = os.environ.get("BASS_MOE_C", "exact")
# the warm tile is only [P, 128] now, so its memset is ~0.1us (the tile
# scheduler requires every read tile to have a writer, so 0 is not allowed)
WARM_INIT = int(os.environ.get("BASS_MOE_WARM_INIT", "1"))

_prog_cache: dict = {}


def _np_dt(name):
    if name == "bfloat16":
        import ml_dtypes

        return ml_dtypes.bfloat16
    return np.float32


def _n_tiles(C):
    """Split C into tiles of at most NSPLIT (remainder split evenly)."""
    out = []
    rem = C
    n0 = 0
    while rem > 0:
        if NT_MODE == "aligned":
            if rem > NSPLIT + NMIN // 2:
                sz = NSPLIT
            elif rem > NSPLIT:
                sz = (rem // 2 + P - 1) // P * P
            else:
                sz = rem
        else:
            if rem > 2 * NSPLIT:
                sz = NSPLIT
            elif rem > NSPLIT and rem - NMIN <= NSPLIT:
                # Make the LAST tile exactly NMIN wide: its eviction
                # (CAST + DMA) is the post-stream critical path, and any
                # width >= NMIN streams at the same per-column rate.
                sz = rem - NMIN
            elif rem > NSPLIT:
                sz = (rem + 1) // 2
            else:
                sz = rem
        out.append((n0, sz))
        n0 += sz
        rem -= sz
    return out


def _build_program(D: int, C: int, mm_dt_name: str, out_dt_name: str):
    import concourse.bacc as bacc
    import concourse.mybir as mybir
    import concourse.tile as tile

    mm_dt = getattr(mybir.dt, mm_dt_name)
    out_dt = getattr(mybir.dt, out_dt_name)
    KT = D // P  # k tiles (contraction)
    MT = D // P  # m tiles (output features)
    KC = KT // PASSES  # k tiles per chunk
    n_tiles = _n_tiles(C)
    NT = len(n_tiles)

    nc = bacc.Bacc(None, target_bir_lowering=False)
    xt = nc.declare_dram_parameter("xt", [NT, P, KT * NSPLIT], mm_dt, isOutput=False)
    w = nc.declare_dram_parameter("w", [P, KT * D], mm_dt, isOutput=False)
    yt = nc.declare_dram_parameter("yt", [MT, P, C], out_dt, isOutput=True)

    with tile.TileContext(nc) as tc:
        with (
            tc.tile_pool(name="wpool", bufs=PASSES + 1) as wpool,
            tc.tile_pool(name="xpool", bufs=PASSES) as xpool,
            tc.tile_pool(name="psum", bufs=8, space="PSUM") as psum_pool,
            tc.tile_pool(name="opool", bufs=4) as opool,
            tc.tile_pool(name="warm", bufs=1) as warmpool,
        ):
            if WARMUP_MM:
                # Keep the PE busy from preamble-end so the HAM clock gate is
                # 8/8 when the real matmuls start.  128-wide for fine-grained
                # overrun; the operands are uninitialized SBUF (the product
                # lands in PSUM tiles nothing reads, and every real
                # accumulation group opens with start=True).
                wt = warmpool.tile([P, P], mybir.dt.bfloat16, tag="warm_w")
                if WARM_INIT:
                    nc.vector.memset(wt[:], 0.0)
                for i in range(WARMUP_MM):
                    wp = psum_pool.tile([P, NSPLIT], mybir.dt.float32, tag="ps")
                    nc.tensor.matmul(
                        wp[:, :P], lhsT=wt[:], rhs=wt[:], start=True, stop=True
                    )

            # --- input DMA triggers ---
            # Input rides the two HWDGE rings so each k-chunk pair (W-c on
            # sync's ring, x0-c on scalar's ring) is delivered by an
            # independent hardware path with parallel descriptor generation;
            # per-ring FIFO keeps chunks in consumption order.
            n0_first, nsz_first = n_tiles[0]
            half = D // 2

            w0a = wpool.tile([P, KC, half], mm_dt, tag="w0a")
            nc.sync.dma_start(w0a[:].rearrange("p k d -> p (k d)"), w[:, 0:half])
            x0_sb = []
            tx = xpool.tile([P, KC, nsz_first], mm_dt, tag="x0_sb")
            nc.scalar.dma_start(
                tx[:].rearrange("p k n -> p (k n)"), xt[0, :, 0 : KC * nsz_first]
            )
            x0_sb.append((tx, 0, 1))
            w0b = wpool.tile([P, KC, half], mm_dt, tag="w0b")
            nc.sync.dma_start(w0b[:].rearrange("p k d -> p (k d)"), w[:, half:D])

            w_sb = [None]
            for c in range(1, PASSES):
                tw = wpool.tile([P, KC, D], mm_dt, tag="w_sb")
                nc.sync.dma_start(
                    tw[:].rearrange("p k d -> p (k d)"),
                    w[:, c * KC * D : (c + 1) * KC * D],
                )
                w_sb.append(tw)
            # Remaining x0 k-tiles ride in PAIRS: a [P, 2, 512] slice has
            # 2KB-contiguous rows, so its DMA packets are 2KB and move at
            # ~320GB/s instead of the 1KB-packet ~190GB/s.  This frees the
            # scalar ring ~1.6us earlier, exactly the window where late W
            # chunks otherwise gap the stream.  c0 stays a single k so the
            # first matmul's data gate stays 128KB.
            x0_pairs = [(1, 2), (3, 2), (5, 2), (7, 1)]
            for k0, nk in x0_pairs:
                tx = xpool.tile([P, nk, nsz_first], mm_dt, tag="x0_sb")
                nc.scalar.dma_start(
                    tx[:].rearrange("p k n -> p (k n)"),
                    xt[0, :, k0 * nsz_first : (k0 + nk) * nsz_first],
                )
                x0_sb.append((tx, k0, nk))
            # j>=1 x tiles ride the sync HWDGE ring BEHIND every W chunk
            # (per-ring FIFO), so their bulk can never steal engine bandwidth
            # from still-pending W chunks.  On gpsimd's software ring they
            # starved both HWDGE rings outright (v3); on the scalar ring they
            # ran ahead of need and starved W c4..c7 (v4).
            x_sb = [None]
            for j, (n0, nsz) in enumerate(n_tiles):
                if j == 0:
                    continue
                t = xpool.tile([P, KT, nsz], mm_dt, tag="x_sb")
                nc.sync.dma_start(
                    t[:].rearrange("p k n -> p (k n)"),
                    xt[j, :, : KT * nsz],
                )
                x_sb.append(t)

            def w_slice(k, mi):
                if k // KC == 0:
                    m0 = mi * P
                    if m0 < half:
                        return w0a[:, k % KC, m0 : m0 + P]
                    return w0b[:, k % KC, m0 - half : m0 - half + P]
                return w_sb[k // KC][:, k % KC, mi * P : (mi + 1) * P]

            def x0_rhs(k, nsz):
                for t, k0, nk in x0_sb:
                    if k0 <= k < k0 + nk:
                        return t[:, k - k0, :nsz]
                raise AssertionError(k)

            def mm(ps, mi, j, k, nsz, start, stop):
                if j == 0:
                    rhs = x0_rhs(k, nsz)
                else:
                    rhs = x_sb[j][:, k, :nsz]
                nc.tensor.matmul(
                    ps[:, :nsz],
                    lhsT=w_slice(k, mi),
                    rhs=rhs,
                    start=start,
                    stop=stop,
                )

            evict_n = [0]

            def evict(ps, mi, n0, nsz):
                ot = opool.tile([P, NSPLIT], out_dt, tag="ot")
                nc.vector.tensor_copy(ot[:, :nsz], ps[:, :nsz])
                evict_n[0] += 1
                if OUT_Q == "alt":
                    q = nc.sync if evict_n[0] % 2 == 1 else nc.scalar
                else:
                    q = nc.sync
                q.dma_start(yt[mi, :, n0 : n0 + nsz], ot[:, :nsz])

            # First n-tile: PASSES k-passes across all m, tracking the
            # arriving W/X0 chunks; evict on the last pass.
            ps0 = []
            for c in range(PASSES):
                for mi in range(MT):
                    if c == 0:
                        ps = psum_pool.tile([P, NSPLIT], mybir.dt.float32, tag="ps")
                        ps0.append(ps)
                    ps = ps0[mi]
                    for k in range(c * KC, (c + 1) * KC):
                        mm(
                            ps,
                            mi,
                            0,
                            k,
                            nsz_first,
                            start=(k == 0),
                            stop=(k == KT - 1),
                        )
                    if c == PASSES - 1:
                        evict(ps, mi, n0_first, nsz_first)
            # Remaining n-tiles: fused k loop.
            for j, (n0, nsz) in enumerate(n_tiles):
                if j == 0:
                    continue
                for mi in range(MT):
                    ps = psum_pool.tile([P, NSPLIT], mybir.dt.float32, tag="ps")
                    for k in range(KT):
                        mm(ps, mi, j, k, nsz, start=(k == 0), stop=(k == KT - 1))
                    evict(ps, mi, n0, nsz)
            if WARMUP_MM and TAIL_MM:
                tp = psum_pool.tile([P, NSPLIT], mybir.dt.float32, tag="ps")
                for i in range(TAIL_MM):
                    nc.tensor.matmul(
                        tp[:, :P], lhsT=wt[:], rhs=wt[:], start=True, stop=True
                    )
    nc.compile()
    return nc


def kernel(hidden: np.ndarray, gate_logits: np.ndarray, W: np.ndarray) -> np.ndarray:
    from concourse.bass_utils import run_bass_kernel_spmd

    hidden = np.asarray(hidden)
    gate_logits = np.asarray(gate_logits)
    W = np.asarray(W)
    B, S, D = hidden.shape
    T, E = gate_logits.shape
    assert E == N_CORES
    x = np.ascontiguousarray(hidden.reshape(T, D).astype(np.float32))

    # --- routing on host (fp32, matches reference softmax/top-2) ---
    g = gate_logits.astype(np.float32)
    m = g.max(axis=-1, keepdims=True)
    p = np.exp(g - m)
    p /= p.sum(axis=-1, keepdims=True)
    top2 = np.argpartition(-p, 1, axis=-1)[:, :2]

    routed = [np.nonzero((top2 == e).any(axis=1))[0] for e in range(E)]
    counts = np.array([len(r) for r in routed])
    if C_MODE == "exact":
        C = max(NMIN, int(counts.max()))
    else:
        C = max(NMIN, int(-(-counts.max() // P)) * P)

    mm_np = _np_dt(MM_DT)
    KT = D // P
    n_tiles = _n_tiles(C)
    NT = len(n_tiles)

    in_maps = []
    for e in range(E):
        idx = routed[e]
        scale = p[idx, e].astype(np.float32)
        xe = x[idx] * scale[:, None]  # [cnt, D]
        xt_full = np.zeros((D, C), dtype=mm_np)
        xt_full[:, : len(idx)] = xe.T.astype(mm_np)
        # [D, C] -> [KT, P, C] -> per-n-tile [NT, P, KT, nsz] contiguous
        xk = xt_full.reshape(KT, P, C)
        xt_dram = np.zeros((NT, P, KT * NSPLIT), dtype=mm_np)
        for j, (n0, nsz) in enumerate(n_tiles):
            blk = xk[:, :, n0 : n0 + nsz].transpose(1, 0, 2)  # [P, KT, nsz]
            xt_dram[j, :, : KT * nsz] = blk.reshape(P, KT * nsz)
        w_full = (
            W[e].astype(mm_np).reshape(KT, P, D).transpose(1, 0, 2).reshape(P, KT * D)
        )
        in_maps.append(
            {
                "xt": np.ascontiguousarray(xt_dram),
                "w": np.ascontiguousarray(w_full),
            }
        )

    key = (D, C, MM_DT, OUT_DT, WARMUP_MM, TAIL_MM, OUT_Q, NT_MODE, WARM_INIT)
    if key not in _prog_cache:
        _prog_cache[key] = _build_program(D, C, MM_DT, OUT_DT)
    nc = _prog_cache[key]

    res = run_bass_kernel_spmd(nc, in_maps, core_ids=list(range(N_CORES)))

    # --- combine on host ---
    out = np.zeros((T, D), dtype=np.float32)
    for e in range(E):
        idx = routed[e]
        ye_t = res.results[e]["yt"].reshape(D, C)  # Y^T
        out[idx] += ye_t[:, : len(idx)].T.astype(np.float32)
    return out.reshape(B, S, D)
